# revision 1
# baseline (speedup 1.0000x reference)
"""MultiHeadAttention (causal + ALiBi) Trainium2 kernel, 8-core SPMD.

Sharding: core c -> batch b = c // 4, head-group j = c % 4 owning global
heads {j, j+4, j+8, j+12} (strided so every core gets one head from each
slope class). Each core projects q/k/v for its 4 heads from x[b], runs
windowed-causal attention in a transposed layout (scores^T[j_kv, i_q]),
and emits a partial out-projection [S, D]. Host sums the 4 partials per
batch (the "all-reduce") and returns [B, S, D].

Math notes:
- ALiBi bias slope*(j-i): the -slope*i part is constant per softmax row
  and cancels; the slope*j part is per-partition in the scores^T layout
  and rides the ACT exp bias input. Blocks are re-centered per i-chunk
  (bias slope*(j - M_it)) to bound exp's dynamic range; the common
  factor cancels in num/l.
- Head slots use per-slot i-chunk widths W (64 for the steepest heads,
  256 otherwise) so that slope*(W-1) stays within fp32's exp range, and
  a j-window (ALiBi locality) skips blocks with negligible weight.
- Matmuls run in float32r (full-rate fp32 variant, ~1.5e-4 rel err) for
  W>=256 and plain fp32 for the W=64 slot (same cost at N<256).
"""
import math
from contextlib import ExitStack

import numpy as np

import concourse.bass as bass
import concourse.tile as tile
from concourse import bacc, mybir
from concourse.bass_utils import run_bass_kernel_spmd

B, S, D, H, HD = 2, 2048, 1024, 16, 64
N_CORES = 8
DT = mybir.dt
F32, F32R = DT.float32, DT.float32r
NEG = -1.0e30

SLOT_W = [64, 256, 256, 256]           # i-chunk width per head slot
SLOT_WIN = [248, 992, 10 ** 9, 10 ** 9]  # j-window per slot (uniform = max over slot heads)
SLOT_DT = [F32R, F32R, F32R, F32R]     # matmul dtype for scores/PV per slot


def slot_blocks(slot):
    """(it, jt, o) list, uniform across cores. o = i0 - 128*jt."""
    W, win = SLOT_W[slot], SLOT_WIN[slot]
    blocks = []
    for it in range(S // W):
        i0 = it * W
        jt_max = (i0 + W - 1) // 128
        jt_min = max(0, math.ceil((i0 - win - 127) / 128))
        for jt in range(jt_min, jt_max + 1):
            blocks.append((it, jt, i0 - 128 * jt))
    return blocks


def slot_offsets(slot):
    """Sorted distinct o values for a slot (bias tile index space)."""
    return sorted({o for _, _, o in slot_blocks(slot)})


def build_nc(repeat=1):
    nc = bacc.Bacc(
        "TRN2", target_bir_lowering=False, debug=False,
        enable_asserts=False, num_devices=N_CORES,
    )
    dram = {}

    def din(name, shape, dtype):
        dram[name] = nc.dram_tensor(name, shape, dtype, kind="ExternalInput").ap()
        return dram[name]

    xT = din("xT", [D, S], F32R)
    wqT = din("wqT", [D, 256], F32R)
    wkT = din("wkT", [D, 256], F32R)
    wvT = din("wvT", [D, 256], F32R)
    bq_p = din("bq_p", [2, 128, 1], F32)
    bk_p = din("bk_p", [2, 128, 1], F32)
    masks_a = din("masks_a", [2, 128, 64], F32)
    masks_b = din("masks_b", [2, 128, 256], F32)
    nbtot = sum(len(slot_offsets(s)) for s in range(4))
    bias_all = din("bias_all", [128, nbtot], F32)
    wout_ab = din("wout_ab", [128, D], F32R)
    wout_cd = din("wout_cd", [128, D], F32R)
    yconst = din("yconst", [1, D], F32)
    y_out = nc.dram_tensor("y", [S, D], F32, kind="ExternalOutput").ap()

    with tile.TileContext(nc) as tc:
        for _ in range(repeat):
            build_body(tc, dram, y_out)
    nc.compile()
    return nc


def build_body(tc, dram, y_out):
    nc = tc.nc
    Exp = mybir.ActivationFunctionType.Exp
    with ExitStack() as ctx:
        consts = ctx.enter_context(tc.tile_pool(name="consts", bufs=1))
        qkpool = ctx.enter_context(tc.tile_pool(name="qk", bufs=1))
        vpool = ctx.enter_context(tc.tile_pool(name="vp", bufs=1))
        attnp = ctx.enter_context(tc.tile_pool(name="attn", bufs=1))
        xtp = ctx.enter_context(tc.tile_pool(name="xt", bufs=12))
        wp = ctx.enter_context(tc.tile_pool(name="w", bufs=8))
        rowp = ctx.enter_context(tc.tile_pool(name="rows", bufs=1))
        prp = ctx.enter_context(tc.tile_pool(name="probs", bufs=20))
        lp = ctx.enter_context(tc.tile_pool(name="lvec", bufs=4))
        rbp = ctx.enter_context(tc.tile_pool(name="rbc", bufs=3))
        yp = ctx.enter_context(tc.tile_pool(name="ysb", bufs=4))
        # PSUM budget (8 banks): big(qkv+y)=3, sc=3, pv=2
        big_ps = ctx.enter_context(tc.tile_pool(name="big_ps", bufs=3, space="PSUM"))
        sc_ps = ctx.enter_context(tc.tile_pool(name="sc_ps", bufs=3, space="PSUM"))
        pv_ps = ctx.enter_context(tc.tile_pool(name="pv_ps", bufs=2, space="PSUM"))

        # ---- persistent q/k/v/attn tiles ----
        # q/k stored as slot-pair tiles [128, S]: slot s lives in partition
        # half (s % 2) of pair tile s // 2
        q_p = [qkpool.tile([128, S], F32R, tag=f"qp{i}", name=f"qp{i}") for i in range(2)]
        k_p = [qkpool.tile([128, S], F32R, tag=f"kp{i}", name=f"kp{i}") for i in range(2)]
        # V' [128, 16 j-tiles, 4 slots, 65]: 64 value cols + ones col
        v_all = vpool.tile([128, 16, 4, 65], F32R, tag="vall", name="vall")
        attn_sb = [attnp.tile([128, S], F32R, tag=f"attn{i}", name=f"attn{i}") for i in range(2)]

        # ---- phase-A weights (DMA-emitted first: on the startup critical path)
        w_sb = {}
        for nm, dr, eng in (("q", "wqT", nc.sync), ("k", "wkT", nc.gpsimd),
                            ("v", "wvT", nc.gpsimd)):
            w_sb[nm] = []
            for kt in range(8):
                t = wp.tile([128, 256], F32R, tag=f"w{nm}", name=f"w{nm}")
                eng.dma_start(out=t[:], in_=dram[dr][kt * 128:(kt + 1) * 128, :])
                w_sb[nm].append(t)
        bpair = {}
        for nm, dr in (("q", "bq_p"), ("k", "bk_p")):
            tiles = []
            for ft in range(2):
                t = rowp.tile([128, 1], F32, tag=f"b{nm}{ft}", name=f"b{nm}{ft}")
                nc.sync.dma_start(out=t[:], in_=dram[dr][ft])
                tiles.append(t)
            bpair[nm] = tiles
        # ones columns of V' (memset; bitcast the f32r view to f32)
        nc.vector.memset(v_all[:, :, :, 64:65].bitcast(F32), 1.0)

        # ---- constants (needed from attention onward) ----
        mask_a_sb, mask_b_sb = [], []
        for mi in range(2):
            t = consts.tile([128, 64], F32, tag=f"maska{mi}", name=f"maska{mi}")
            nc.sync.dma_start(out=t[:], in_=dram["masks_a"][mi])
            mask_a_sb.append(t)
            t = consts.tile([128, 256], F32, tag=f"maskb{mi}", name=f"maskb{mi}")
            nc.sync.dma_start(out=t[:], in_=dram["masks_b"][mi])
            mask_b_sb.append(t)
        nbtot = sum(len(slot_offsets(s)) for s in range(4))
        bias_tile = consts.tile([128, nbtot], F32, tag="bias_all", name="bias_all")
        nc.sync.dma_start(out=bias_tile[:], in_=dram["bias_all"])
        bias_sb = []
        col = 0
        for s in range(4):
            d = {}
            for o in slot_offsets(s):
                d[o] = bias_tile[:, col:col + 1]
                col += 1
            bias_sb.append(d)
        wout_sb = []
        for nm in ("wout_ab", "wout_cd"):
            t = consts.tile([128, D], F32R, tag=nm, name=nm)
            nc.sync.dma_start(out=t[:], in_=dram[nm])
            wout_sb.append(t)
        yconst_bc = consts.tile([128, D], F32, tag="yconst_bc", name="yconst_bc")
        yconst_bcast = bass.AP(
            tensor=dram["yconst"].tensor, offset=0, ap=[[0, 128], [1, D]])
        nc.sync.dma_start(out=yconst_bc[:], in_=yconst_bcast)

        by_slot = []
        for s in range(4):
            by_it = {}
            for it, jt, o in slot_blocks(s):
                by_it.setdefault(it, []).append((jt, o))
            by_slot.append(by_it)

        def emit_proj(ch):
            """Load x^T chunk ch and project q/k/v for its 512 tokens."""
            xt = []
            for kt in range(8):
                t = xtp.tile([128, 512], F32R, tag="xt", name="xt")
                nc.scalar.dma_start(
                    out=t[:],
                    in_=dram["xT"][kt * 128:(kt + 1) * 128, ch * 512:(ch + 1) * 512])
                xt.append(t)
            sl = slice(ch * 512, (ch + 1) * 512)
            for nm, dst in (("q", q_p), ("k", k_p)):
                for ft in range(2):      # feature pair (slots 2ft, 2ft+1)
                    ps = big_ps.tile([128, 512], F32, tag="big", name="qkv")
                    for kt in range(8):
                        nc.tensor.matmul(
                            ps[:], w_sb[nm][kt][:, ft * 128:(ft + 1) * 128],
                            xt[kt][:], start=(kt == 0), stop=(kt == 7))
                    nc.vector.tensor_scalar_add(
                        dst[ft][:, sl], ps[:], bpair[nm][ft][:])
            for tl in range(4):
                tt = ch * 4 + tl
                ps = big_ps.tile([128, 512], F32, tag="big", name="qkvv")
                for kt in range(8):
                    nc.tensor.matmul(
                        ps[:, 0:256], xt[kt][:, tl * 128:(tl + 1) * 128],
                        w_sb["v"][kt][:], start=(kt == 0), stop=(kt == 7))
                nc.vector.tensor_copy(
                    v_all[:, tt:tt + 1, :, 0:64],
                    ps[:, 0:256].rearrange("p (a b) -> p a b", a=4))

        def emit_scores(s, it):
            """Scores+mask+exp for one chunk; returns probs list."""
            W, dt_s = SLOT_W[s], SLOT_DT[s]
            prs = []
            h0 = (s % 2) * 64
            kp_s = k_p[s // 2]
            qp_s = q_p[s // 2]
            for jt, o in by_slot[s][it]:
                sc = sc_ps.tile([128, W], F32, tag="sc", name="sc")
                nc.tensor.matmul(
                    sc[:], kp_s[h0:h0 + 64, jt * 128:(jt + 1) * 128],
                    qp_s[h0:h0 + 64, it * W:(it + 1) * W],
                    start=True, stop=True)
                if o <= 127:  # diagonal block -> causal mask add
                    if s == 0:
                        msk = mask_a_sb[o // 64]
                    else:
                        msk = mask_b_sb[0 if o == 0 else 1]
                    nc.vector.tensor_add(sc[:], sc[:], msk[:])
                ptag = "pr_a" if s == 0 else "pr_b"
                pr = prp.tile([128, W], dt_s, tag=ptag, name="pr", bufs=(12 if s == 0 else 20))
                nc.scalar.activation(pr[:], sc[:], Exp, bias=bias_sb[s][o][:])
                prs.append((jt, pr))
            return prs

        def emit_pv(s, it, prs):
            """PV accumulation + normalize epilogue for one chunk."""
            W = SLOT_W[s]
            pv = pv_ps.tile([65, W], F32, tag="pv", name="pv")
            for bi, (jt, pr) in enumerate(prs):
                nc.tensor.matmul(
                    pv[:], v_all[:, jt:jt + 1, s:s + 1, :], pr[:],
                    start=(bi == 0), stop=(bi == len(prs) - 1))
            rr = lp.tile([1, W], F32, tag="rr", name="rr")
            nc.vector.reciprocal(rr[:], pv[64:65, :])
            rb = rbp.tile([64, W], F32, tag="rb", name="rb")
            nc.gpsimd.partition_broadcast(rb[:], rr[:])
            dst = attn_sb[s // 2]
            r0 = (s % 2) * 64
            nc.vector.tensor_mul(
                dst[r0:r0 + 64, it * W:(it + 1) * W], pv[0:64, :], rb[:])

        def emit_yproj(tt):
            """Out-projection for token tile tt (needs attn rows complete)."""
            for oc in range(2):
                py = big_ps.tile([128, 512], F32, tag="big", name="py")
                nc.tensor.matmul(
                    py[:], attn_sb[0][:, tt * 128:(tt + 1) * 128],
                    wout_sb[0][:, oc * 512:(oc + 1) * 512],
                    start=True, stop=False)
                nc.tensor.matmul(
                    py[:], attn_sb[1][:, tt * 128:(tt + 1) * 128],
                    wout_sb[1][:, oc * 512:(oc + 1) * 512],
                    start=False, stop=True)
                ysb = yp.tile([128, 512], F32, tag="ysb", name="ysb")
                nc.vector.tensor_add(ysb[:], py[:], yconst_bc[:, oc * 512:(oc + 1) * 512])
                nc.scalar.dma_start(
                    out=y_out[tt * 128:(tt + 1) * 128, oc * 512:(oc + 1) * 512],
                    in_=ysb[:])

        # ---- fused schedule: per 512-token chunk: project -> attention -> yproj
        # software pipeline: scores of chunk i emitted before pv of chunk i-1
        prev = None
        pending_y = []
        for ch in range(4):
            emit_proj(ch)
            a0 = ch * 8          # slot-A chunks in this ch (W=64): a0..a0+7
            b0 = ch * 2          # slot B/C/D chunks in this ch (W=256): b0, b0+1
            chunks = [
                (2, b0), (0, a0), (0, a0 + 1),
                (3, b0), (0, a0 + 2), (0, a0 + 3),
                (1, b0), (0, a0 + 4),
                (2, b0 + 1), (0, a0 + 5),
                (3, b0 + 1), (0, a0 + 6),
                (1, b0 + 1), (0, a0 + 7),
            ]
            for s, it in chunks:
                cur = (s, it, emit_scores(s, it))
                if prev is not None:
                    emit_pv(*prev)
                prev = cur
            # out-proj for the PREVIOUS chunk's tokens (its attn rows are done
            # once this chunk's pv for all slots completes; emit with 1-chunk lag)
            if ch > 0:
                for tt in range((ch - 1) * 4, ch * 4):
                    pending_y.append(tt)
            while len(pending_y) > 2:
                emit_yproj(pending_y.pop(0))
        emit_pv(*prev)
        for tt in pending_y + list(range(12, 16)):
            emit_yproj(tt)


def make_in_maps(x, w_qkv, b_qkv, w_out, b_out):
    """Host-side sharding + constant prep. Returns list of 8 in_maps."""
    x = np.asarray(x, np.float32)
    w_qkv = np.asarray(w_qkv, np.float32)
    b_qkv = np.asarray(b_qkv, np.float32)
    w_out = np.asarray(w_out, np.float32)
    b_out = np.asarray(b_out, np.float32)

    slopes = (2.0 ** (-(np.arange(1, H + 1)) * 8.0 / H)).astype(np.float64)

    # shared constants
    masks_a = np.empty((2, 128, 64), np.float32)
    for mi, o in enumerate((0, 64)):
        p = np.arange(128)[:, None]
        f = np.arange(64)[None, :]
        masks_a[mi] = np.where(p <= o + f, 0.0, NEG)
    masks_b = np.empty((2, 128, 256), np.float32)
    for mi, o in enumerate((0, -128)):
        p = np.arange(128)[:, None]
        f = np.arange(256)[None, :]
        masks_b[mi] = np.where(p <= o + f, 0.0, NEG)

    in_maps = []
    for c in range(N_CORES):
        b, j = divmod(c, 4)
        heads = [j, j + 4, j + 8, j + 12]
        cols = np.concatenate([np.arange(h * HD, (h + 1) * HD) for h in heads])
        wq = w_qkv[cols, :] / 8.0                  # [256, 1024], scale folded
        wk = w_qkv[D + cols, :]
        wv = w_qkv[2 * D + cols, :]
        bq = b_qkv[cols] / 8.0
        bk = b_qkv[D + cols]
        bv = b_qkv[2 * D + cols]
        w_out_loc = w_out[:, cols]                  # [1024, 256]
        yconst = (w_out_loc @ bv + b_out / 4.0).astype(np.float32)[None, :]

        cols = []
        for s in range(4):
            Wl = SLOT_W[s]
            sl = slopes[heads[s]]
            for o in slot_offsets(s):
                cols.append(sl * (np.arange(128) - o - Wl + 1))
        bias_all = np.stack(cols, axis=1).astype(np.float32)

        in_maps.append(dict(
            xT=np.ascontiguousarray(x[b].T),
            wqT=np.ascontiguousarray(wq.T),
            wkT=np.ascontiguousarray(wk.T),
            wvT=np.ascontiguousarray(wv.T),
            bq_p=np.ascontiguousarray(bq.reshape(2, 128, 1)),
            bk_p=np.ascontiguousarray(bk.reshape(2, 128, 1)),
            masks_a=masks_a, masks_b=masks_b, bias_all=bias_all,
            wout_ab=np.ascontiguousarray(w_out_loc[:, 0:128].T),
            wout_cd=np.ascontiguousarray(w_out_loc[:, 128:256].T),
            yconst=yconst,
        ))
    return in_maps


_NC_CACHE = {}


def _get_nc(repeat=1):
    if repeat not in _NC_CACHE:
        _NC_CACHE[repeat] = build_nc(repeat)
    return _NC_CACHE[repeat]


def kernel(x, w_qkv, b_qkv, w_out, b_out, block_mask=None):
    in_maps = make_in_maps(x, w_qkv, b_qkv, w_out, b_out)
    nc = _get_nc(1)
    res = run_bass_kernel_spmd(nc, in_maps, list(range(N_CORES)), trace=False)
    y = np.zeros((B, S, D), np.float64)
    for c in range(N_CORES):
        y[c // 4] += res.results[c]["y"].astype(np.float64)
    return y.astype(np.float32)



# revision 7
# speedup vs baseline: 3.4724x; 3.4724x over previous
"""MultiHeadAttention (causal + ALiBi) Trainium2 kernel, 8-core SPMD.

Sharding: core c -> batch b = c // 4, head-group j = c % 4 owning global
heads {j, j+4, j+8, j+12} (strided so every core gets one head from each
slope class). Each core projects q/k/v for its 4 heads from x[b], runs
windowed-causal attention in a transposed layout (scores^T[j_kv, i_q]),
and emits a partial out-projection [S, D] in bf16. Host sums the 4
partials per batch plus the w_out@b_v+b_out constant and returns [B,S,D].

Design notes:
- All attention matmuls in bf16 (PE runs 1 cycle/row at any free size;
  fp32r needs >=256). x / qkv weights / out weights also bf16.
- Per-slot q-chunk widths W = [128, 256, 512, 512] and ALiBi kv windows
  SLOT_WIN = [127, 384, inf, inf]: steep heads keep 2 kv blocks, slot 1
  keeps 5 (dropped tail mass < 1e-4 of kept), slots 2-3 run full causal.
- ALiBi exp bias recentered to the CHUNK MIDDLE: slope*(j - o - (W-1)/2)
  bounds exp overflow AND the softmax-denominator underflow, making
  W=128 safe for the steepest head (slope 2^-0.5).
- Causal mask adds cover only the diagonal band (width 128*(mi+1)).
- PV in the v1 layout pv[65, W] = [v|1]^T @ probs; reciprocal +
  partition_broadcast + multiply normalizes into attn_sb.
- Fat DMAs: x/w pre-reshaped host-side to [128, 8k, X] so each chunk
  loads in 2 DMAs (each DMA costs ~600ns SEQ + ~630ns shared HWDGE).
- Schedule: proj(ch+1) pieces are interleaved into attention(ch)
  emission; yproj fires per token-tile as soon as its 4 slots finish;
  y DMAs lag a few tiles so they never dep-block the ACT queue.
- PSUM banks: big(qkv+yproj)=2, sc(scores)=4, pv=2.
"""
import math
from contextlib import ExitStack

import ml_dtypes
import numpy as np

import concourse.bass as bass
import concourse.tile as tile
from concourse import bacc, mybir
from concourse.bass_utils import run_bass_kernel_spmd

B, S, D, H, HD = 2, 2048, 1024, 16, 64
N_CORES = 8
DT = mybir.dt
F32, F32R, BF16 = DT.float32, DT.float32r, DT.bfloat16
NEG = -1.0e30

SLOT_W = [128, 256, 512, 512]          # i-chunk width per head slot
SLOT_WIN = [127, 384, 10 ** 9, 10 ** 9]  # j-window per slot (uniform = max over slot heads)
SLOT_DT = [BF16, BF16, BF16, BF16]     # matmul dtype for scores/PV per slot


def slot_blocks(slot):
    """(it, jt, o) list, uniform across cores. o = i0 - 128*jt."""
    W, win = SLOT_W[slot], SLOT_WIN[slot]
    blocks = []
    for it in range(S // W):
        i0 = it * W
        jt_max = (i0 + W - 1) // 128
        jt_min = max(0, math.ceil((i0 - win - 127) / 128))
        for jt in range(jt_min, jt_max + 1):
            blocks.append((it, jt, i0 - 128 * jt))
    return blocks


def slot_offsets(slot):
    """Sorted distinct o values for a slot (bias tile index space)."""
    return sorted({o for _, _, o in slot_blocks(slot)})


def build_nc(repeat=1):
    nc = bacc.Bacc(
        "TRN2", target_bir_lowering=False, debug=False,
        enable_asserts=False, num_devices=N_CORES,
    )
    dram = {}

    def din(name, shape, dtype):
        dram[name] = nc.dram_tensor(name, shape, dtype, kind="ExternalInput").ap()
        return dram[name]

    nbtot = sum(len(slot_offsets(s)) for s in range(4))
    din("xT", [128, 8 * S], BF16)
    din("wqT", [128, 8 * 256], BF16)
    din("wkT", [128, 8 * 256], BF16)
    din("wvT", [128, 8 * 256], BF16)
    # packed consts: 2 bq cols | mask_a 128 | masks_b 512 | bias nbtot
    din("call", [128, 642 + 2048 + nbtot], F32)
    din("wout", [128, 2 * D], BF16)
    y_out = nc.dram_tensor("y", [S, D], BF16, kind="ExternalOutput").ap()

    with tile.TileContext(nc) as tc:
        for _ in range(repeat):
            build_body(tc, dram, y_out)
    nc.compile()
    return nc


def build_body(tc, dram, y_out):
    nc = tc.nc
    Exp = mybir.ActivationFunctionType.Exp
    with ExitStack() as ctx:
        consts = ctx.enter_context(tc.tile_pool(name="consts", bufs=1))
        qkpool = ctx.enter_context(tc.tile_pool(name="qk", bufs=1))
        vpool = ctx.enter_context(tc.tile_pool(name="vp", bufs=1))
        attnp = ctx.enter_context(tc.tile_pool(name="attn", bufs=1))
        xtp = ctx.enter_context(tc.tile_pool(name="xt", bufs=3))
        wp = ctx.enter_context(tc.tile_pool(name="w", bufs=1))
        rowp = ctx.enter_context(tc.tile_pool(name="rows", bufs=1))
        prp = ctx.enter_context(tc.tile_pool(name="probs", bufs=28))
        lp = ctx.enter_context(tc.tile_pool(name="lvec", bufs=4))
        rbp = ctx.enter_context(tc.tile_pool(name="rbc", bufs=3))
        yp = ctx.enter_context(tc.tile_pool(name="ysb", bufs=10))
        # PSUM budget (8 banks): big(qkv+y)=3, sc=3, pv=2
        big_ps = ctx.enter_context(tc.tile_pool(name="big_ps", bufs=2, space="PSUM"))
        sc_ps = ctx.enter_context(tc.tile_pool(name="sc_ps", bufs=4, space="PSUM"))
        pv_ps = ctx.enter_context(tc.tile_pool(name="pv_ps", bufs=2, space="PSUM"))

        # ---- persistent q/k/v/attn tiles ----
        # q/k stored as slot-pair tiles [128, S]: slot s lives in partition
        # half (s % 2) of pair tile s // 2
        q_p = [qkpool.tile([128, S], BF16, tag=f"qp{i}", name=f"qp{i}") for i in range(2)]
        k_p = [qkpool.tile([128, S], BF16, tag=f"kp{i}", name=f"kp{i}") for i in range(2)]
        # V' [128, 16 j-tiles, 4 slots, 65]: 64 value cols + ones col
        v_all = vpool.tile([128, 16, 4, 65], BF16, tag="vall", name="vall")
        attn_sb = [attnp.tile([128, S], BF16, tag=f"attn{i}", name=f"attn{i}") for i in range(2)]

        # ---- phase-A weights: fat DMAs (2 halves q/k for pipelining) ----
        w_sb = {}
        for nm, dr in (("q", "wqT"), ("k", "wkT")):
            t = wp.tile([128, 8, 256], BF16, tag=f"w{nm}", name=f"w{nm}")
            wr = dram[dr].rearrange("p (k f) -> p k f", k=8)
            nc.sync.dma_start(out=t[:, 0:4, :], in_=wr[:, 0:4, :])
            nc.sync.dma_start(out=t[:, 4:8, :], in_=wr[:, 4:8, :])
            w_sb[nm] = t
        t = wp.tile([128, 8, 256], BF16, tag="wv", name="wv")
        nc.sync.dma_start(out=t[:], in_=dram["wvT"].rearrange("p (k f) -> p k f", k=8))
        w_sb["v"] = t
        # ones columns of V'
        nc.vector.memset(v_all[:, :, :, 64:65], 1.0)

        # ---- packed constants (one DMA): bq | mask_a | masks_b | bias ----
        nbtot = sum(len(slot_offsets(s)) for s in range(4))
        call = consts.tile([128, 642 + 2048 + nbtot], F32, tag="call", name="call")
        nc.sync.dma_start(out=call[:, 0:642], in_=dram["call"][:, 0:642])
        nc.sync.dma_start(out=call[:, 2690:2690 + nbtot],
                          in_=dram["call"][:, 2690:2690 + nbtot])
        nc.sync.dma_start(out=call[:, 642:2690], in_=dram["call"][:, 642:2690])
        bpair = {"q": [call[:, 0:1], call[:, 1:2]]}
        mask_a_sb = [call[:, 2:130]]
        mask_b_sb = [call[:, 130:386], call[:, 386:642]]
        mask_c_sb = [call[:, 642 + 512 * i:642 + 512 * (i + 1)] for i in range(4)]
        bias_sb = []
        col = 642 + 2048
        for s in range(4):
            d = {}
            for o in slot_offsets(s):
                d[o] = call[:, col:col + 1]
                col += 1
            bias_sb.append(d)
        wout_t = consts.tile([128, 2 * D], BF16, tag="wout", name="wout")
        wout_sb = [wout_t[:, 0:D], wout_t[:, D:2 * D]]

        def emit_wout_loads():
            nc.sync.dma_start(out=wout_t[:], in_=dram["wout"])

        by_slot = []
        for s in range(4):
            by_it = {}
            for it, jt, o in slot_blocks(s):
                by_it.setdefault(it, []).append((jt, o))
            by_slot.append(by_it)

        def emit_xt(ch):
            """Issue x^T chunk loads (2 fat DMAs); returns the chunk tile."""
            t = xtp.tile([128, 8, 512], BF16, tag="xt", name="xt")
            xr = dram["xT"].rearrange("p (k t) -> p k t", k=8)
            sl = slice(ch * 512, (ch + 1) * 512)
            nc.scalar.dma_start(out=t[:, 0:4, :], in_=xr[:, 0:4, sl])
            nc.scalar.dma_start(out=t[:, 4:8, :], in_=xr[:, 4:8, sl])
            return t

        def proj_pieces(ch, xt):
            """Projection work split into 8 independently-emittable pieces."""
            sl = slice(ch * 512, (ch + 1) * 512)

            def qk_piece(nm, dst, ft):
                def go():
                    ps = big_ps.tile([128, 512], F32, tag="big", name="qkv")
                    for kt in range(8):
                        nc.tensor.matmul(
                            ps[:], w_sb[nm][:, kt, ft * 128:(ft + 1) * 128],
                            xt[:, kt, :], start=(kt == 0), stop=(kt == 7))
                    if nm == "q":
                        nc.vector.tensor_scalar_add(
                            dst[ft][:, sl], ps[:], bpair["q"][ft])
                    else:
                        nc.vector.tensor_copy(dst[ft][:, sl], ps[:])
                return go

            def v_piece(tl):
                def go():
                    tt = ch * 4 + tl
                    ps = big_ps.tile([128, 512], F32, tag="big", name="qkvv")
                    for kt in range(8):
                        nc.tensor.matmul(
                            ps[:, 0:256], xt[:, kt, tl * 128:(tl + 1) * 128],
                            w_sb["v"][:, kt, :], start=(kt == 0), stop=(kt == 7))
                    nc.vector.tensor_copy(
                        v_all[:, tt:tt + 1, :, 0:64],
                        ps[:, 0:256].rearrange("p (a b) -> p a b", a=4))
                return go

            return [qk_piece("q", q_p, 0), qk_piece("k", k_p, 0),
                    qk_piece("q", q_p, 1), qk_piece("k", k_p, 1),
                    v_piece(0), v_piece(1), v_piece(2), v_piece(3)]

        def emit_scores(s, it):
            """Scores+mask+exp for one chunk; returns probs list."""
            W, dt_s = SLOT_W[s], SLOT_DT[s]
            prs = []
            h0 = (s % 2) * 64
            kp_s = k_p[s // 2]
            qp_s = q_p[s // 2]
            for jt, o in by_slot[s][it]:
                sc = sc_ps.tile([128, W], F32, tag="sc", name="sc")
                nc.tensor.matmul(
                    sc[:], kp_s[h0:h0 + 64, jt * 128:(jt + 1) * 128],
                    qp_s[h0:h0 + 64, it * W:(it + 1) * W],
                    start=True, stop=True)
                if o <= 127:  # diagonal block -> causal mask add
                    # only the diagonal band needs masking: past col
                    # 128*(mi+1)-1 every kv row is causally valid
                    if s == 0:
                        msk, mw = mask_a_sb[0], 128
                    elif s == 1:
                        mi = 0 if o == 0 else 1
                        msk, mw = mask_b_sb[mi], 128 * (mi + 1)
                    else:
                        mi = (-o) // 128
                        msk, mw = mask_c_sb[mi], 128 * (mi + 1)
                    nc.vector.tensor_add(sc[:, 0:mw], sc[:, 0:mw], msk[:, 0:mw])
                ptag = "pr_a" if s == 0 else "pr_b"
                pr = prp.tile([128, W], dt_s, tag=ptag, name="pr", bufs=(12 if s == 0 else 20))
                nc.scalar.activation(pr[:], sc[:], Exp, bias=bias_sb[s][o][:])
                prs.append((jt, pr))
            return prs

        def emit_pv(s, it, prs):
            """PV accumulation + normalize epilogue for one chunk."""
            W = SLOT_W[s]
            pv = pv_ps.tile([65, W], F32, tag="pv", name="pv")
            for bi, (jt, pr) in enumerate(prs):
                nc.tensor.matmul(
                    pv[:], v_all[:, jt:jt + 1, s:s + 1, :], pr[:],
                    start=(bi == 0), stop=(bi == len(prs) - 1))
            rr = lp.tile([1, W], F32, tag="rr", name="rr")
            nc.vector.reciprocal(rr[:], pv[64:65, :])
            rb = rbp.tile([64, W], F32, tag="rb", name="rb")
            nc.gpsimd.partition_broadcast(rb[:], rr[:])
            dst = attn_sb[s // 2]
            r0 = (s % 2) * 64
            nc.vector.tensor_mul(
                dst[r0:r0 + 64, it * W:(it + 1) * W], pv[0:64, :], rb[:])

        ydma_q = []

        def flush_ydma(keep=3):
            while len(ydma_q) > keep:
                tt, ysb = ydma_q.pop(0)
                nc.scalar.dma_start(
                    out=y_out[tt * 128:(tt + 1) * 128, :], in_=ysb[:])

        def emit_yproj(tt):
            """Out-projection for token tile tt (needs attn rows complete)."""
            ysb = yp.tile([128, 1024], BF16, tag="ysb", name="ysb")
            for oc in range(2):
                py = big_ps.tile([128, 512], F32, tag="big", name="py")
                nc.tensor.matmul(
                    py[:], attn_sb[0][:, tt * 128:(tt + 1) * 128],
                    wout_sb[0][:, oc * 512:(oc + 1) * 512],
                    start=True, stop=False)
                nc.tensor.matmul(
                    py[:], attn_sb[1][:, tt * 128:(tt + 1) * 128],
                    wout_sb[1][:, oc * 512:(oc + 1) * 512],
                    start=False, stop=True)
                nc.vector.tensor_copy(ysb[:, oc * 512:(oc + 1) * 512], py[:])
            ydma_q.append((tt, ysb))

        # ---- schedule: attention(ch) interleaved with proj(ch+1) pieces;
        # yproj emitted as soon as a token tile's four slots are normalized
        slot_done = [set() for _ in range(4)]
        pending_y = list(range(16))

        def flush_ready(limit=2):
            n = 0
            while pending_y and n < limit:
                tt = pending_y[0]
                if (tt in slot_done[0] and tt // 2 in slot_done[1]
                        and tt // 4 in slot_done[2] and tt // 4 in slot_done[3]):
                    emit_yproj(pending_y.pop(0))
                    n += 1
                else:
                    break

        prev = None
        xt = emit_xt(0)
        pieces = proj_pieces(0, xt)
        for p in pieces:
            p()
        for ch in range(4):
            a0, b0 = ch * 4, ch * 2
            chunks = [
                (0, a0), (1, b0), (2, ch), (0, a0 + 1), (3, ch),
                (0, a0 + 2), (0, a0 + 3), (1, b0 + 1),
            ] if ch == 0 else ([
                (2, ch), (0, a0), (3, ch), (0, a0 + 1),
                (1, b0), (0, a0 + 2), (0, a0 + 3), (1, b0 + 1),
            ] if ch < 3 else [
                (2, ch), (3, ch), (1, b0), (0, a0), (0, a0 + 1),
                (1, b0 + 1), (0, a0 + 2), (0, a0 + 3),
            ])
            pieces = []
            for ci, (s, it) in enumerate(chunks):
                if ci == 3:
                    if ch == 0:
                        emit_wout_loads()
                    if ch < 3:
                        xt = emit_xt(ch + 1)
                        pieces = list(proj_pieces(ch + 1, xt))
                cur = (s, it, emit_scores(s, it))
                if prev is not None:
                    emit_pv(*prev)
                    slot_done[prev[0]].add(prev[1])
                prev = cur
                if pieces:
                    pieces.pop(0)()
                    if len(pieces) > 6 - ci:
                        pieces.pop(0)()
                flush_ready()
                flush_ydma()
            for p in pieces:
                p()
        emit_pv(*prev)
        slot_done[prev[0]].add(prev[1])
        flush_ready(limit=16)
        while pending_y:
            emit_yproj(pending_y.pop(0))
        flush_ydma(keep=0)


def make_in_maps(x, w_qkv, b_qkv, w_out, b_out):
    """Host-side sharding + constant prep. Returns list of 8 in_maps."""
    x = np.asarray(x, np.float32)
    w_qkv = np.asarray(w_qkv, np.float32)
    b_qkv = np.asarray(b_qkv, np.float32)
    w_out = np.asarray(w_out, np.float32)
    b_out = np.asarray(b_out, np.float32)

    slopes = (2.0 ** (-(np.arange(1, H + 1)) * 8.0 / H)).astype(np.float64)

    # shared constants
    masks_a = np.empty((1, 128, 128), np.float32)
    p = np.arange(128)[:, None]
    f = np.arange(128)[None, :]
    masks_a[0] = np.where(p <= f, 0.0, NEG)
    masks_c = np.empty((4, 128, 512), np.float32)
    f512 = np.arange(512)[None, :]
    for mi in range(4):
        masks_c[mi] = np.where(p <= f512 - 128 * mi, 0.0, NEG)
    masks_b = np.empty((2, 128, 256), np.float32)
    for mi, o in enumerate((0, -128)):
        p = np.arange(128)[:, None]
        f = np.arange(256)[None, :]
        masks_b[mi] = np.where(p <= o + f, 0.0, NEG)

    in_maps = []
    yconsts = []
    for c in range(N_CORES):
        b, j = divmod(c, 4)
        heads = [j, j + 4, j + 8, j + 12]
        cols = np.concatenate([np.arange(h * HD, (h + 1) * HD) for h in heads])
        wq = w_qkv[cols, :] / 8.0                  # [256, 1024], scale folded
        wk = w_qkv[D + cols, :]
        wv = w_qkv[2 * D + cols, :]
        bq = b_qkv[cols] / 8.0
        bk = b_qkv[D + cols]
        bv = b_qkv[2 * D + cols]
        w_out_loc = w_out[:, cols]                  # [1024, 256]
        yconsts.append(w_out_loc @ bv)

        cols = []
        for s in range(4):
            Wl = SLOT_W[s]
            sl = slopes[heads[s]]
            for o in slot_offsets(s):
                cols.append(sl * (np.arange(128) - o - (Wl - 1) / 2.0))
        bias_all = np.stack(cols, axis=1).astype(np.float32)

        call = np.concatenate([
            bq.reshape(2, 128).T, masks_a[0], masks_b[0], masks_b[1],
            masks_c[0], masks_c[1], masks_c[2], masks_c[3], bias_all,
        ], axis=1).astype(np.float32)
        in_maps.append(dict(
            xT=np.ascontiguousarray(x[b].T.reshape(8, 128, S).transpose(1, 0, 2).reshape(128, 8 * S)).astype(ml_dtypes.bfloat16),
            wqT=np.ascontiguousarray(wq.T.reshape(8, 128, 256).transpose(1, 0, 2).reshape(128, 8 * 256)).astype(ml_dtypes.bfloat16),
            wkT=np.ascontiguousarray(wk.T.reshape(8, 128, 256).transpose(1, 0, 2).reshape(128, 8 * 256)).astype(ml_dtypes.bfloat16),
            wvT=np.ascontiguousarray(wv.T.reshape(8, 128, 256).transpose(1, 0, 2).reshape(128, 8 * 256)).astype(ml_dtypes.bfloat16),
            call=call,
            wout=np.ascontiguousarray(w_out_loc.T.reshape(2, 128, D).transpose(
                1, 0, 2).reshape(128, 2 * D)).astype(ml_dtypes.bfloat16),
        ))
    yconst = [
        (yconsts[0] + yconsts[1] + yconsts[2] + yconsts[3] + b_out).astype(np.float32),
        (yconsts[4] + yconsts[5] + yconsts[6] + yconsts[7] + b_out).astype(np.float32),
    ]
    return in_maps, yconst


_NC_CACHE = {}


def _get_nc(repeat=1):
    if repeat not in _NC_CACHE:
        _NC_CACHE[repeat] = build_nc(repeat)
    return _NC_CACHE[repeat]


def kernel(x, w_qkv, b_qkv, w_out, b_out, block_mask=None):
    in_maps, yconst = make_in_maps(x, w_qkv, b_qkv, w_out, b_out)
    nc = _get_nc(1)
    res = run_bass_kernel_spmd(nc, in_maps, list(range(N_CORES)), trace=False)
    y = np.zeros((B, S, D), np.float64)
    for c in range(N_CORES):
        y[c // 4] += res.results[c]["y"].astype(np.float64)
    y[0] += yconst[0]
    y[1] += yconst[1]
    return y.astype(np.float32)

